# revision 9
# baseline (speedup 1.0000x reference)
"""Trainium2 Bass kernel for nn_DeChunkLayer.

Per batch row (one NeuronCore each, pure data parallel):
  1. gate[c]: boundary-sorted clipped probabilities (host, tiny).
  2. EMA linear recurrence over chunks h_c = (1-g_c) h_{c-1} + g_c x_c as a
     blocked lower-triangular matmul "scan": for each 128-chunk block t,
       ema_t = L_t @ X_t (+ lookback term)
     with L entries g_j * prod(1-g_k) host-computed in f64 log space.
     Because the decay product over >=64 chunks underflows far below fp32
     resolution for these gates, each block is computed INDEPENDENTLY from a
     host-verified lookback window of LB preceding chunks (no serial carry
     chain). If the decay bound ever fails, falls back to an exact
     carry-chain formulation (cp (x) h_prev rank-1 matmul per block).
  3. Dechunk out[s] = ema[cid[s]] as one-hot selection matmuls per 128-token
     block. Selection matrices are 0/1 so they ship as fp8e4 (exact),
     host-built: no on-device index math, and the dechunk only depends on
     the sel DMA + the ema blocks it reads.

Engine plan: PE streams scan+dechunk matmuls back-to-back; DVE/ACT each
copy half of every PSUM block to SBUF; x loads ride SWDGE (gpsimd) while
lt/lt2 (sync ring) and sel (scalar ring) load in parallel; output groups
alternate the two HWDGE rings. ema rows are stored partition-reversed per
block so the carry row is partition 0 (compute engines need 32-aligned
partition bases).
"""

import math

import numpy as np
import ml_dtypes

import concourse.bacc as bacc
import concourse.mybir as mybir
from concourse import tile
from concourse.bass_utils import run_bass_kernel_spmd

B, SEQ, MAXC, DIM = 8, 4096, 2048, 1024
BLK = 128
NCORES = 8
NTB = SEQ // BLK  # 32 token blocks
F32 = mybir.dt.float32
F16 = mybir.dt.float16
F8 = mybir.dt.float8e4
NWARM = 3  # warmup matmuls per half (PE activity while input DMA streams)
# output staging group sizes (token blocks per out DMA); tapered tail so the
# final DMA after the last matmul is small
GRPS = [2, 2, 2, 2, 4, 4, 4, 4, 4, 2, 2]
assert sum(GRPS) == NTB and all(g % 2 == 0 for g in GRPS)


def _preprocess(chunk_states, boundary_mask, boundary_prob):
    """Host-side index/gate math.

    Returns (in_maps, NBLK, windows, LB) where LB>0 selects the lookback
    scan (LB in {64,128}) and LB=0 selects the carry-chain fallback.
    """
    chunk_states = np.asarray(chunk_states, dtype=np.float32)
    boundary_mask = np.asarray(boundary_mask)
    boundary_prob = np.asarray(boundary_prob, dtype=np.float32)

    p_full = np.clip(boundary_prob[..., -1], np.float32(1e-4), np.float32(1.0 - 1e-4))
    token_idx = np.arange(SEQ)[None, :] + (~boundary_mask).astype(np.int32) * SEQ
    order = np.argsort(token_idx, axis=1, kind="stable")
    gate = np.take_along_axis(p_full, order[:, :MAXC], axis=1)  # [B, C]

    cid = np.cumsum(boundary_mask.astype(np.int32), axis=1) - 1  # [B, S]
    cid = np.clip(cid, 0, MAXC - 1)
    n_used = int(cid.max()) + 1
    NBLK = max(1, math.ceil(n_used / BLK))
    CU = NBLK * BLK

    g = gate[:, :CU].astype(np.float64)
    a = 1.0 - g
    S = np.cumsum(np.log(a), axis=1)  # [B, CU] global log-decay prefix

    # pick the smallest lookback window whose dropped prefix is negligible
    LB = 0
    for cand in (64, 128):
        ok = True
        for t in range(1, NBLK):
            j0 = t * BLK - cand - 1
            if j0 < 0:
                continue  # window reaches chunk 0: nothing dropped
            if np.any(S[:, t * BLK] - S[:, j0] > -18.0):
                ok = False
                break
        if ok:
            LB = cand
            break

    ii = np.arange(BLK)[:, None]
    jj = np.arange(BLK)[None, :]
    Sb = S.reshape(B, NBLK, BLK)
    # main (within-block) coefficients: L[b,t,i,j] = g_j exp(S_i - S_j), i>=j
    Lf = np.where(
        ii[None, None] >= jj[None, None],
        np.exp(Sb[:, :, :, None] - Sb[:, :, None, :])
        * g.reshape(B, NBLK, 1, BLK),
        0.0,
    )
    # ema rows stored partition-reversed (chunk i -> partition 127-i)
    Lf = Lf[:, :, ::-1, :]
    LT_sb = np.ascontiguousarray(
        Lf.transpose(0, 3, 1, 2).reshape(B, BLK, NBLK * BLK).astype(np.float16)
    )

    # lookback coefficients: for block t>=1, chunk jb=(t-1)*128+j feeding
    # out chunk t*128+i:  g_jb exp(S[t*128+i] - S[jb]), only j >= 128-LB.
    # Shipped trimmed to the LB used partitions.
    LBr = max(LB, 1)
    lt2_sb = np.zeros((B, LBr, NBLK * BLK), dtype=np.float16)
    if LB > 0:
        for t in range(1, NBLK):
            Sout = S[:, t * BLK:(t + 1) * BLK]  # [B, 128]
            Sin = S[:, t * BLK - LB:t * BLK]  # [B, LB]
            gin = g[:, t * BLK - LB:t * BLK]
            Lb = np.exp(Sout[:, None, :] - Sin[:, :, None]) * gin[:, :, None]
            # out chunk i -> partition 127-i  => reverse the i axis
            lt2_sb[:, :, t * BLK:(t + 1) * BLK] = Lb[:, :, ::-1].astype(
                np.float16
            )

    # carry-chain fallback data: cp[t,i] = prod_{k<=i in block} a_k, reversed
    ls_blk = np.cumsum(np.log(a).reshape(B, NBLK, BLK), axis=2)
    cp = np.exp(ls_blk).astype(np.float16)[:, :, ::-1]
    cp_sb = np.ascontiguousarray(cp.reshape(B, 1, NBLK * BLK))

    # dechunk union windows per token block
    cidr = cid.reshape(B, NTB, BLK)
    lo = (cidr[:, :, 0] // BLK).min(axis=0)  # [NTB]
    hi = (cidr[:, :, -1] // BLK).max(axis=0)
    windows = [list(range(int(lo[tb]), int(hi[tb]) + 1)) for tb in range(NTB)]
    ncols = sum(len(w) for w in windows)

    # host-built 0/1 selection matrices, fp8e4 (exact for 0/1):
    # sel[b, p, col*128 + j] = (cid[b, tb*128+j] == t*128 + 127 - p)
    pidx = np.arange(BLK)
    sel = np.zeros((B, BLK, ncols * BLK), dtype=np.float16)
    col = 0
    for tb in range(NTB):
        toks = cid[:, tb * BLK:(tb + 1) * BLK]  # [B, 128]
        for t in windows[tb]:
            chunk_of_p = t * BLK + (BLK - 1 - pidx)  # [128]
            eq = toks[:, None, :] == chunk_of_p[None, :, None]  # [B,128,128]
            sel[:, :, col * BLK:(col + 1) * BLK] = eq
            col += 1

    in_maps = []
    for b in range(B):
        in_maps.append(
            {
                "x": np.ascontiguousarray(
                    chunk_states[b, :CU]
                    .astype(np.float16)
                    .reshape(NBLK, BLK, DIM)
                    .transpose(1, 0, 2)
                    .reshape(BLK, NBLK * DIM)
                ),
                "lt": LT_sb[b],
                "lt2": np.ascontiguousarray(lt2_sb[b]),
                "cp": cp_sb[b],
                "sel": np.ascontiguousarray(sel[b]),
            }
        )
    return in_maps, NBLK, windows, LB


def _build_nc(NBLK, windows, LB):
    ncols = sum(len(w) for w in windows)
    LBr = max(LB, 1)
    nc = bacc.Bacc("TRN2", target_bir_lowering=False, debug=False, num_devices=8)
    x = nc.dram_tensor("x", [BLK, NBLK * DIM], F16, kind="ExternalInput")
    lt = nc.dram_tensor("lt", [BLK, NBLK * BLK], F16, kind="ExternalInput")
    lt2 = nc.dram_tensor("lt2", [LBr, NBLK * BLK], F16, kind="ExternalInput")
    cp = nc.dram_tensor("cp", [1, NBLK * BLK], F16, kind="ExternalInput")
    sel = nc.dram_tensor("sel", [BLK, ncols * BLK], F16, kind="ExternalInput")
    out = nc.dram_tensor("out", [SEQ, DIM], F16, kind="ExternalOutput")

    # sel pieces: cols for tbs [0,6), [6,16), [16,32). Later pieces are
    # issued mid-scan so their (large-descriptor) transfers don't starve
    # the x stream in the DMA round-robin.
    cum = [0]
    for tb in range(NTB):
        cum.append(cum[-1] + len(windows[tb]))
    sc1, sc2 = cum[min(6, NTB)], cum[min(16, NTB)]
    sel_cuts = [(0, sc1), (sc1, sc2), (sc2, ncols)]

    with tile.TileContext(nc) as tc:
        with (
            tc.tile_pool(name="const", bufs=1) as const_pool,
            tc.tile_pool(name="outp", bufs=4) as outpool,
            tc.tile_pool(name="pp", bufs=2, space="PSUM") as pp_pool,
        ):
            # sync ring: lt + x stream + even out groups; scalar ring:
            # sel pieces + lt2 + odd out groups
            lt_sb = const_pool.tile([BLK, NBLK * BLK], F16, tag="lt")
            nc.sync.dma_start(lt_sb[:], lt[:])
            x_sb = const_pool.tile([BLK, NBLK * DIM], F16, tag="x")
            xcuts = sorted({min(c, NBLK) for c in (1, 2, 3, 5, 7, 9)} | {NBLK})
            c_prev = 0
            for c1 in xcuts:
                nc.sync.dma_start(
                    x_sb[:, c_prev * DIM:c1 * DIM], x[:, c_prev * DIM:c1 * DIM]
                )
                c_prev = c1
            sel_sb = const_pool.tile([BLK, ncols * BLK], F16, tag="sel")
            nc.scalar.dma_start(
                sel_sb[:, sel_cuts[0][0] * BLK:sel_cuts[0][1] * BLK],
                sel[:, sel_cuts[0][0] * BLK:sel_cuts[0][1] * BLK],
            )
            lt2_sb = None
            if LB > 0:
                # full-height tile, data landed at partitions [BLK-LB, BLK)
                # so the lookback lhsT base matches the x rhs base
                lt2_sb = const_pool.tile([BLK, NBLK * BLK], F16, tag="lt2")
                nc.scalar.dma_start(lt2_sb[BLK - LB:BLK, :], lt2[:])
            cp_sb = const_pool.tile([1, NBLK * BLK], F16, tag="cp")
            if LB == 0:
                nc.scalar.dma_start(cp_sb[:], cp[:])
            ema = const_pool.tile([BLK, NBLK * DIM], F16, tag="ema")

            # PE warmup: zero-weight matmuls accumulating into the first
            # scan tile (add 0, cannot be dead-code-eliminated). PE
            # activity releases the HAM clock throttle before real work.
            zw = const_pool.tile([BLK, BLK], F16, tag="zw")
            nc.gpsimd.memset(zw[:], 0.0)
            zx = const_pool.tile([BLK, 512], F16, tag="zx")
            nc.gpsimd.memset(zx[:], 0.0)
            ps0 = pp_pool.tile([BLK, 2 * DIM], F32, tag="pp", name="pp_s0")
            for k in range(NWARM):
                for h in range(2):
                    nc.tensor.matmul(
                        ps0[:, h * 512:(h + 1) * 512], lhsT=zw[:], rhs=zx[:],
                        start=(k == 0), stop=False,
                    )

            def paired_copy(dst2, src2, nblks):
                """PSUM->SBUF copy of nblks (1 or 2) DIM-wide blocks, split
                across DVE and ACT; paired blocks use one strided op each."""
                if nblks == 2:
                    s3 = src2.rearrange("p (b d) -> p b d", b=2)
                    d3 = dst2.rearrange("p (b d) -> p b d", b=2)
                    nc.vector.tensor_copy(out=d3[:, :, :512], in_=s3[:, :, :512])
                    nc.scalar.copy(out=d3[:, :, 512:], in_=s3[:, :, 512:])
                else:
                    nc.vector.tensor_copy(
                        out=dst2[:, :512], in_=src2[:, :512]
                    )
                    nc.scalar.copy(out=dst2[:, 512:], in_=src2[:, 512:])

            # ---- dechunk emitter (pairs of token blocks per psum tile) ----
            state = {"col": 0, "tb": 0, "gi": 0}

            def emit_group(grp):
                gi = state["gi"]
                og = outpool.tile([BLK, grp * DIM], F16, tag=f"og{grp}",
                                  name=f"og_{gi}")
                for i in range(0, grp, 2):
                    tb0 = state["tb"]
                    po = pp_pool.tile([BLK, 2 * DIM], F32, tag="pp",
                                      name=f"pp_o{tb0}")
                    for j in range(2):
                        tb = tb0 + j
                        col = state["col"]
                        w = windows[tb]
                        for wi, t in enumerate(w):
                            for h in range(2):
                                nc.tensor.matmul(
                                    po[:, j * DIM + h * 512:
                                       j * DIM + (h + 1) * 512],
                                    lhsT=sel_sb[:, (col + wi) * BLK:
                                                (col + wi + 1) * BLK],
                                    rhs=ema[:, t * DIM + h * 512:
                                            t * DIM + (h + 1) * 512],
                                    start=(wi == 0),
                                    stop=(wi == len(w) - 1),
                                )
                        state["col"] = col + len(w)
                        state["tb"] = tb + 1
                    paired_copy(og[:, i * DIM:(i + 2) * DIM], po[:], 2)
                tb_lo = state["tb"] - grp
                dma_eng = nc.sync if (gi % 2) == 0 else nc.scalar
                dma_eng.dma_start(
                    out[tb_lo * BLK:state["tb"] * BLK, :].rearrange(
                        "(i p) d -> p i d", p=BLK
                    ),
                    og[:].rearrange("p (i d) -> p i d", d=DIM),
                )
                state["gi"] = gi + 1

            # a group is ready once the last ema block it reads is copied;
            # with paired scan copies block t lands with its pair partner
            def pair_end(t):
                if LB == 0:
                    return t
                return min(t | 1, NBLK - 1)

            group_need = []
            tb = 0
            for grp in GRPS:
                need = max(max(windows[t]) for t in range(tb, tb + grp))
                group_need.append(pair_end(need))
                tb += grp

            def scan_block(t, ps, joff):
                for h in range(2):
                    sl = slice(joff * DIM + h * 512, joff * DIM + (h + 1) * 512)
                    xsl = slice(t * DIM + h * 512, t * DIM + (h + 1) * 512)
                    nc.tensor.matmul(
                        ps[:, sl],
                        lhsT=lt_sb[:, t * BLK:(t + 1) * BLK],
                        rhs=x_sb[:, xsl],
                        start=(t != 0),
                        stop=(t == 0),
                    )
                    if t > 0:
                        if LB > 0:
                            p0 = BLK - LB
                            lsl = slice((t - 1) * DIM + h * 512,
                                        (t - 1) * DIM + (h + 1) * 512)
                            nc.tensor.matmul(
                                ps[:, sl],
                                lhsT=lt2_sb[p0:BLK, t * BLK:(t + 1) * BLK],
                                rhs=x_sb[p0:BLK, lsl],
                                start=False,
                                stop=True,
                            )
                        else:
                            # carry chain: cp_t (x) h_prev, h_prev = row 0 of
                            # the previous block's (reversed) fp16 ema
                            esl = slice((t - 1) * DIM + h * 512,
                                        (t - 1) * DIM + (h + 1) * 512)
                            nc.tensor.matmul(
                                ps[:, sl],
                                lhsT=cp_sb[:, t * BLK:(t + 1) * BLK],
                                rhs=ema[0:1, esl],
                                start=False,
                                stop=True,
                            )

            def post_pair(t_done):
                # stage later sel pieces once the x stream has priority'd
                if t_done >= 3 and state.get("sel2") is None:
                    a, b = sel_cuts[1]
                    if b > a:
                        nc.scalar.dma_start(
                            sel_sb[:, a * BLK:b * BLK], sel[:, a * BLK:b * BLK]
                        )
                    state["sel2"] = True
                if t_done >= 5 and state.get("sel3") is None:
                    a, b = sel_cuts[2]
                    if b > a:
                        nc.scalar.dma_start(
                            sel_sb[:, a * BLK:b * BLK], sel[:, a * BLK:b * BLK]
                        )
                    state["sel3"] = True
                while (state["gi"] < len(GRPS)
                       and group_need[state["gi"]] <= t_done):
                    emit_group(GRPS[state["gi"]])

            state["sel2"] = None if NBLK > 3 else True
            state["sel3"] = None if NBLK > 5 else True
            if state["sel2"] is True or state["sel3"] is True:
                # tiny NBLK: load everything upfront
                a, b = sel_cuts[0][1], ncols
                if b > a:
                    nc.scalar.dma_start(
                        sel_sb[:, a * BLK:b * BLK], sel[:, a * BLK:b * BLK]
                    )
                state["sel2"] = state["sel3"] = True

            # ---- blocked matmul scan, two chunk blocks per psum tile ----
            if LB > 0:
                t = 0
                while t < NBLK:
                    npair = 2 if t + 1 < NBLK else 1
                    ps = ps0 if t == 0 else pp_pool.tile(
                        [BLK, 2 * DIM], F32, tag="pp", name=f"pp_s{t}"
                    )
                    for j in range(npair):
                        scan_block(t + j, ps, j)
                    paired_copy(
                        ema[:, t * DIM:(t + npair) * DIM],
                        ps[:, :npair * DIM], npair,
                    )
                    t += npair
                    post_pair(t - 1)
            else:
                # carry chain is serial: per-block copies
                for t in range(NBLK):
                    ps = ps0 if t == 0 else pp_pool.tile(
                        [BLK, 2 * DIM], F32, tag="pp", name=f"pp_s{t}"
                    )
                    scan_block(t, ps, 0)
                    paired_copy(
                        ema[:, t * DIM:(t + 1) * DIM], ps[:, :DIM], 1
                    )
                    post_pair(t)

            while state["gi"] < len(GRPS):
                emit_group(GRPS[state["gi"]])

    nc.finalize()
    return nc


def _run(in_maps, NBLK, windows, LB):
    nc = _build_nc(NBLK, windows, LB)
    res = run_bass_kernel_spmd(nc, in_maps, core_ids=list(range(NCORES)))
    return np.stack(
        [res.results[i]["out"].astype(np.float32) for i in range(NCORES)], axis=0
    )


def kernel(chunk_states, boundary_mask, boundary_prob):
    in_maps, NBLK, windows, LB = _preprocess(
        chunk_states, boundary_mask, boundary_prob
    )
    last_err = None
    for _ in range(3):  # retry transient accelerator failures
        try:
            return _run(in_maps, NBLK, windows, LB)
        except Exception as e:  # noqa: BLE001
            last_err = e
            try:
                import jax

                jax.clear_caches()
            except Exception:  # noqa: BLE001
                pass
    raise last_err


# revision 12
# speedup vs baseline: 1.2901x; 1.2901x over previous
"""Trainium2 Bass kernel for nn_DeChunkLayer.

Per batch row (one NeuronCore each, pure data parallel):
  1. gate[c]: boundary-sorted clipped probabilities (host, tiny).
  2. EMA linear recurrence over chunks h_c = (1-g_c) h_{c-1} + g_c x_c as a
     blocked lower-triangular matmul "scan": for each 128-chunk block t,
       ema_t = L_t @ X_t (+ lookback term)
     with L entries g_j * prod(1-g_k) host-computed in f64 log space.
     Because the decay product over >=64 chunks underflows far below fp32
     resolution for these gates, each block is computed INDEPENDENTLY from a
     host-verified lookback window of LB preceding chunks (no serial carry
     chain). If the decay bound ever fails, falls back to an exact
     carry-chain formulation (cp (x) h_prev rank-1 matmul per block).
  3. Dechunk out[s] = ema[cid[s]] as one-hot selection matmuls per 128-token
     block. Selection matrices are 0/1 so they ship as fp8e4 (exact),
     host-built: no on-device index math, and the dechunk only depends on
     the sel DMA + the ema blocks it reads.

Engine plan: PE streams scan+dechunk matmuls back-to-back; DVE/ACT each
copy half of every PSUM block to SBUF; x loads ride SWDGE (gpsimd) while
lt/lt2 (sync ring) and sel (scalar ring) load in parallel; output groups
alternate the two HWDGE rings. ema rows are stored partition-reversed per
block so the carry row is partition 0 (compute engines need 32-aligned
partition bases).
"""

import math

import numpy as np
import ml_dtypes

import concourse.bacc as bacc
import concourse.mybir as mybir
from concourse import tile
from concourse.bass_utils import run_bass_kernel_spmd

B, SEQ, MAXC, DIM = 8, 4096, 2048, 1024
BLK = 128
NCORES = 8
NTB = SEQ // BLK  # 32 token blocks
F32 = mybir.dt.float32
F16 = mybir.dt.float16
F8 = mybir.dt.float8e4
NWARM = 3  # warmup matmuls per half (PE activity while input DMA streams)
# output staging group sizes (token blocks per out DMA); tapered tail so the
# final DMA after the last matmul is small
GRPS = [1, 1, 2, 2, 4, 4, 4, 4, 4, 2, 2, 1, 1]
assert sum(GRPS) == NTB


def _preprocess(chunk_states, boundary_mask, boundary_prob):
    """Host-side index/gate math.

    Returns (in_maps, NBLK, windows, LB) where LB>0 selects the lookback
    scan (LB in {64,128}) and LB=0 selects the carry-chain fallback.
    """
    chunk_states = np.asarray(chunk_states, dtype=np.float32)
    boundary_mask = np.asarray(boundary_mask)
    boundary_prob = np.asarray(boundary_prob, dtype=np.float32)

    p_full = np.clip(boundary_prob[..., -1], np.float32(1e-4), np.float32(1.0 - 1e-4))
    token_idx = np.arange(SEQ)[None, :] + (~boundary_mask).astype(np.int32) * SEQ
    order = np.argsort(token_idx, axis=1, kind="stable")
    gate = np.take_along_axis(p_full, order[:, :MAXC], axis=1)  # [B, C]

    cid = np.cumsum(boundary_mask.astype(np.int32), axis=1) - 1  # [B, S]
    cid = np.clip(cid, 0, MAXC - 1)
    n_used = int(cid.max()) + 1
    NBLK = max(1, math.ceil(n_used / BLK))
    CU = NBLK * BLK

    g = gate[:, :CU].astype(np.float64)
    a = 1.0 - g
    S = np.cumsum(np.log(a), axis=1)  # [B, CU] global log-decay prefix

    # pick the smallest lookback window whose dropped prefix is negligible
    LB = 0
    for cand in (64, 128):
        ok = True
        for t in range(1, NBLK):
            j0 = t * BLK - cand - 1
            if j0 < 0:
                continue  # window reaches chunk 0: nothing dropped
            if np.any(S[:, t * BLK] - S[:, j0] > -18.0):
                ok = False
                break
        if ok:
            LB = cand
            break

    ii = np.arange(BLK)[:, None]
    jj = np.arange(BLK)[None, :]
    Sb = S.reshape(B, NBLK, BLK)
    # main (within-block) coefficients: L[b,t,i,j] = g_j exp(S_i - S_j), i>=j
    Lf = np.where(
        ii[None, None] >= jj[None, None],
        np.exp(Sb[:, :, :, None] - Sb[:, :, None, :])
        * g.reshape(B, NBLK, 1, BLK),
        0.0,
    )
    # ema rows stored partition-reversed (chunk i -> partition 127-i)
    Lf = Lf[:, :, ::-1, :]
    LT_sb = np.ascontiguousarray(
        Lf.transpose(0, 3, 1, 2).reshape(B, BLK, NBLK * BLK).astype(np.float16)
    )

    # lookback coefficients: for block t>=1, chunk jb=(t-1)*128+j feeding
    # out chunk t*128+i:  g_jb exp(S[t*128+i] - S[jb]), only j >= 128-LB.
    # Shipped trimmed to the LB used partitions.
    LBr = max(LB, 1)
    lt2_sb = np.zeros((B, LBr, NBLK * BLK), dtype=np.float16)
    if LB > 0:
        for t in range(1, NBLK):
            Sout = S[:, t * BLK:(t + 1) * BLK]  # [B, 128]
            Sin = S[:, t * BLK - LB:t * BLK]  # [B, LB]
            gin = g[:, t * BLK - LB:t * BLK]
            Lb = np.exp(Sout[:, None, :] - Sin[:, :, None]) * gin[:, :, None]
            # out chunk i -> partition 127-i  => reverse the i axis
            lt2_sb[:, :, t * BLK:(t + 1) * BLK] = Lb[:, :, ::-1].astype(
                np.float16
            )

    # carry-chain fallback data: cp[t,i] = prod_{k<=i in block} a_k, reversed
    ls_blk = np.cumsum(np.log(a).reshape(B, NBLK, BLK), axis=2)
    cp = np.exp(ls_blk).astype(np.float16)[:, :, ::-1]
    cp_sb = np.ascontiguousarray(cp.reshape(B, 1, NBLK * BLK))

    # dechunk union windows per token block
    cidr = cid.reshape(B, NTB, BLK)
    lo = (cidr[:, :, 0] // BLK).min(axis=0)  # [NTB]
    hi = (cidr[:, :, -1] // BLK).max(axis=0)
    windows = [list(range(int(lo[tb]), int(hi[tb]) + 1)) for tb in range(NTB)]
    ncols = sum(len(w) for w in windows)

    # host-built 0/1 selection matrices, fp8e4 (exact for 0/1):
    # sel[b, p, col*128 + j] = (cid[b, tb*128+j] == t*128 + 127 - p)
    pidx = np.arange(BLK)
    sel = np.zeros((B, BLK, ncols * BLK), dtype=np.float16)
    col = 0
    for tb in range(NTB):
        toks = cid[:, tb * BLK:(tb + 1) * BLK]  # [B, 128]
        for t in windows[tb]:
            chunk_of_p = t * BLK + (BLK - 1 - pidx)  # [128]
            eq = toks[:, None, :] == chunk_of_p[None, :, None]  # [B,128,128]
            sel[:, :, col * BLK:(col + 1) * BLK] = eq
            col += 1

    in_maps = []
    for b in range(B):
        in_maps.append(
            {
                "x": np.ascontiguousarray(
                    chunk_states[b, :CU]
                    .astype(np.float16)
                    .reshape(NBLK, BLK, DIM)
                    .transpose(1, 0, 2)
                    .reshape(BLK, NBLK * DIM)
                ),
                "lt": LT_sb[b],
                "lt2": np.ascontiguousarray(lt2_sb[b]),
                "cp": cp_sb[b],
                "sel": np.ascontiguousarray(sel[b]),
            }
        )
    return in_maps, NBLK, windows, LB


def _build_nc(NBLK, windows, LB):
    ncols = sum(len(w) for w in windows)
    LBr = max(LB, 1)
    NE = (NBLK + 1) // 2  # even-indexed scan blocks
    NO = NBLK // 2        # odd-indexed
    nc = bacc.Bacc("TRN2", target_bir_lowering=False, debug=False, num_devices=8)
    x = nc.dram_tensor("x", [BLK, NBLK * DIM], F16, kind="ExternalInput")
    lt = nc.dram_tensor("lt", [BLK, NBLK * BLK], F16, kind="ExternalInput")
    lt2 = nc.dram_tensor("lt2", [LBr, NBLK * BLK], F16, kind="ExternalInput")
    cp = nc.dram_tensor("cp", [1, NBLK * BLK], F16, kind="ExternalInput")
    sel = nc.dram_tensor("sel", [BLK, ncols * BLK], F16, kind="ExternalInput")
    out = nc.dram_tensor("out", [SEQ, DIM], F16, kind="ExternalOutput")

    # sel pieces: cols for tbs [0,6), [6,16), [16,32). Later pieces are
    # issued mid-scan so their (large-descriptor) transfers don't starve
    # the x stream in the DMA round-robin.
    cum = [0]
    for tb in range(NTB):
        cum.append(cum[-1] + len(windows[tb]))
    sc1, sc2 = cum[min(6, NTB)], cum[min(16, NTB)]
    sel_cuts = [(0, sc1), (sc1, sc2), (sc2, ncols)]

    with tile.TileContext(nc) as tc:
        with (
            tc.tile_pool(name="const", bufs=1) as const_pool,
            tc.tile_pool(name="outp", bufs=4) as outpool,
            tc.tile_pool(name="pp", bufs=4, space="PSUM") as pp_pool,
        ):
            # sync ring: lt + x stream + DVE-copied out groups; scalar
            # ring: sel pieces + lt2 + ACT-copied out groups
            lt_sb = const_pool.tile([BLK, NBLK * BLK], F16, tag="lt")
            nc.sync.dma_start(lt_sb[:], lt[:])
            x_sb = const_pool.tile([BLK, NBLK * DIM], F16, tag="x")
            xcuts = sorted({min(c, NBLK) for c in (1, 2, 3, 5, 7, 9)} | {NBLK})
            c_prev = 0
            for c1 in xcuts:
                nc.sync.dma_start(
                    x_sb[:, c_prev * DIM:c1 * DIM], x[:, c_prev * DIM:c1 * DIM]
                )
                c_prev = c1
            sel_sb = const_pool.tile([BLK, ncols * BLK], F16, tag="sel")
            nc.scalar.dma_start(
                sel_sb[:, sel_cuts[0][0] * BLK:sel_cuts[0][1] * BLK],
                sel[:, sel_cuts[0][0] * BLK:sel_cuts[0][1] * BLK],
            )
            lt2_sb = None
            if LB > 0:
                # full-height tile, data landed at partitions [BLK-LB, BLK)
                # so the lookback lhsT base matches the x rhs base
                lt2_sb = const_pool.tile([BLK, NBLK * BLK], F16, tag="lt2")
                nc.scalar.dma_start(lt2_sb[BLK - LB:BLK, :], lt2[:])
            cp_sb = const_pool.tile([1, NBLK * BLK], F16, tag="cp")
            if LB == 0:
                nc.scalar.dma_start(cp_sb[:], cp[:])
            # ema split by scan-block parity: each tile has exactly ONE
            # writer engine (DVE for even blocks, ACT for odd) — Tile
            # chains same-tile accesses from different engines serially,
            # so sharing one ema tile would serialize the copies.
            ema_ev = const_pool.tile([BLK, NE * DIM], F16, tag="emae",
                                     name="ema_ev")
            ema_od = (const_pool.tile([BLK, NO * DIM], F16, tag="emao",
                                      name="ema_od")
                      if NO else None)

            def ema_slice(t, h0, h1):
                tile_ = ema_ev if t % 2 == 0 else ema_od
                base = (t // 2) * DIM
                return tile_[:, base + h0:base + h1]

            # PE warmup: zero-weight matmuls accumulating into the first
            # scan tile (add 0, cannot be dead-code-eliminated). PE
            # activity releases the HAM clock throttle before real work.
            zw = const_pool.tile([BLK, BLK], F16, tag="zw")
            nc.gpsimd.memset(zw[:], 0.0)
            zx = const_pool.tile([BLK, 512], F16, tag="zx")
            nc.gpsimd.memset(zx[:], 0.0)
            ps0 = pp_pool.tile([BLK, DIM], F32, tag="pp", name="pp_s0")
            for k in range(NWARM):
                for h in range(2):
                    nc.tensor.matmul(
                        ps0[:, h * 512:(h + 1) * 512], lhsT=zw[:], rhs=zx[:],
                        start=(k == 0), stop=False,
                    )

            # ---- dechunk emitter: one copier engine per out group ----
            state = {"col": 0, "tb": 0, "gi": 0}

            def emit_group(grp):
                gi = state["gi"]
                og = outpool.tile([BLK, grp * DIM], F16, tag=f"og{grp}",
                                  name=f"og_{gi}")
                use_dve = (gi % 2) == 0
                for i in range(grp):
                    tb = state["tb"]
                    col = state["col"]
                    w = windows[tb]
                    po = pp_pool.tile([BLK, DIM], F32, tag="pp",
                                      name=f"pp_o{tb}")
                    for wi, t in enumerate(w):
                        for h in range(2):
                            nc.tensor.matmul(
                                po[:, h * 512:(h + 1) * 512],
                                lhsT=sel_sb[:, (col + wi) * BLK:
                                            (col + wi + 1) * BLK],
                                rhs=ema_slice(t, h * 512, (h + 1) * 512),
                                start=(wi == 0),
                                stop=(wi == len(w) - 1),
                            )
                    state["col"] = col + len(w)
                    dst = og[:, i * DIM:(i + 1) * DIM]
                    if use_dve:
                        nc.vector.tensor_copy(out=dst, in_=po[:])
                    else:
                        nc.scalar.copy(out=dst, in_=po[:])
                    state["tb"] = tb + 1
                tb_lo = state["tb"] - grp
                dma_eng = nc.sync if use_dve else nc.scalar
                dma_eng.dma_start(
                    out[tb_lo * BLK:state["tb"] * BLK, :].rearrange(
                        "(i p) d -> p i d", p=BLK
                    ),
                    og[:].rearrange("p (i d) -> p i d", d=DIM),
                )
                state["gi"] = gi + 1

            group_need = []
            tb = 0
            for grp in GRPS:
                group_need.append(max(max(windows[t]) for t in range(tb, tb + grp)))
                tb += grp

            state["sel_loaded"] = sel_cuts[0][1]
            state["sel_piece"] = 1

            def load_sel_through(col_hi):
                # issue staged sel pieces (in order) until cols [0, col_hi)
                # are covered; MUST precede any matmul reading those cols
                while state["sel_loaded"] < col_hi:
                    a, b = sel_cuts[state["sel_piece"]]
                    if b > a:
                        nc.scalar.dma_start(
                            sel_sb[:, a * BLK:b * BLK], sel[:, a * BLK:b * BLK]
                        )
                    state["sel_loaded"] = b
                    state["sel_piece"] += 1

            def post_block(t_done):
                # stage later sel pieces once the x stream has priority'd
                if t_done >= 3:
                    load_sel_through(sel_cuts[1][1])
                if t_done >= 5:
                    load_sel_through(ncols)
                while (state["gi"] < len(GRPS)
                       and group_need[state["gi"]] <= t_done):
                    gi = state["gi"]
                    tb_hi = sum(GRPS[:gi + 1])
                    load_sel_through(cum[tb_hi])
                    emit_group(GRPS[gi])

            # ---- blocked matmul scan over chunk blocks ----
            for t in range(NBLK):
                ps = ps0 if t == 0 else pp_pool.tile(
                    [BLK, DIM], F32, tag="pp", name=f"pp_s{t}"
                )
                for h in range(2):
                    sl = slice(h * 512, (h + 1) * 512)
                    xsl = slice(t * DIM + h * 512, t * DIM + (h + 1) * 512)
                    nc.tensor.matmul(
                        ps[:, sl],
                        lhsT=lt_sb[:, t * BLK:(t + 1) * BLK],
                        rhs=x_sb[:, xsl],
                        start=(t != 0),
                        stop=(t == 0),
                    )
                    if t > 0:
                        if LB > 0:
                            p0 = BLK - LB
                            lsl = slice((t - 1) * DIM + h * 512,
                                        (t - 1) * DIM + (h + 1) * 512)
                            nc.tensor.matmul(
                                ps[:, sl],
                                lhsT=lt2_sb[p0:BLK, t * BLK:(t + 1) * BLK],
                                rhs=x_sb[p0:BLK, lsl],
                                start=False,
                                stop=True,
                            )
                        else:
                            # carry chain: cp_t (x) h_prev, h_prev = row 0
                            # of the previous block's (reversed) fp16 ema
                            nc.tensor.matmul(
                                ps[:, sl],
                                lhsT=cp_sb[:, t * BLK:(t + 1) * BLK],
                                rhs=ema_slice(t - 1, h * 512,
                                              (h + 1) * 512)[0:1, :],
                                start=False,
                                stop=True,
                            )
                # full-width psum -> fp16 ema copy by the tile's one engine
                dst = ema_slice(t, 0, DIM)
                if t % 2 == 0:
                    nc.vector.tensor_copy(out=dst, in_=ps[:])
                else:
                    nc.scalar.copy(out=dst, in_=ps[:])
                post_block(t)

            while state["gi"] < len(GRPS):
                gi = state["gi"]
                load_sel_through(cum[sum(GRPS[:gi + 1])])
                emit_group(GRPS[gi])

    nc.finalize()
    return nc


def _run(in_maps, NBLK, windows, LB):
    nc = _build_nc(NBLK, windows, LB)
    res = run_bass_kernel_spmd(nc, in_maps, core_ids=list(range(NCORES)))
    return np.stack(
        [res.results[i]["out"].astype(np.float32) for i in range(NCORES)], axis=0
    )


def kernel(chunk_states, boundary_mask, boundary_prob):
    in_maps, NBLK, windows, LB = _preprocess(
        chunk_states, boundary_mask, boundary_prob
    )
    last_err = None
    for _ in range(3):  # retry transient accelerator failures
        try:
            return _run(in_maps, NBLK, windows, LB)
        except Exception as e:  # noqa: BLE001
            last_err = e
            try:
                import jax

                jax.clear_caches()
            except Exception:  # noqa: BLE001
                pass
    raise last_err
